# revision 6
# baseline (speedup 1.0000x reference)
"""GQA with sliding-window + global-sink sparse attention on 8 trn2 NeuronCores.

Sharding: batch x head-group. 8 cores = 2 batches x 4 groups.
Group g owns q heads 8g..8g+7 and kv heads 2g, 2g+1 (GQA factor 4 kept local).
No collectives: each core computes a rank-1024 partial of y = O @ Wo for its
batch; the host sums the 4 partials per batch.

On-device layout choices (all matmuls bf16, fp32 PSUM accumulation):
  Q^T, K^T in [d, t] (projection with W chunks as stationary lhsT)
  V in [t, d'] (projection with x^T chunks as stationary lhsT)
  S^T = K @ Q^T flash tiles over the causal band (window 1024 + 32 sinks)
  softmax without max-subtraction (scores bounded ~ +-15)
  row sums via ones-matmul, normalization via K=1 broadcast matmul
RoPE rotate-half is a 64-partition rotation done with partition-offset DVE
copies (verified on HW).
"""
import numpy as np

B, T, D = 2, 2048, 4096
H, KH, HD = 32, 8, 128
WINDOW, NGLOBAL = 1024, 32
TC = 256                       # token chunk (projections and query chunks)
NTC = T // TC                  # 8
SCALE = 1.0 / np.sqrt(HD)
MASK_IDX = {"M1": 0, "M1g": 1, "M2": 2, "M3": 3, "M4": 4}

_CACHE = {}

# run_bass_kernel_spmd kwargs, settable by test harness (e.g. trace)
RUN_KWARGS = {}
LAST_RESULT = None


def _ktile_schedule(m):
    """k-tiles (ks, mask) for query chunk m; plus has_global flag."""
    qs = m * TC
    tiles = []
    if qs < 1024:
        for ks in range(0, qs - 127, 128):
            tiles.append((ks, None))
        tiles.append((qs, "M4"))
        tiles.append((qs + 128, "M3"))
        return tiles, False
    if qs == 1024:
        tiles.append((0, "M1g"))
        tiles.append((128, "M2"))
        for ks in range(256, qs - 127, 128):
            tiles.append((ks, None))
        tiles.append((qs, "M4"))
        tiles.append((qs + 128, "M3"))
        return tiles, False
    tiles.append((qs - 1024, "M1"))
    tiles.append((qs - 896, "M2"))
    for ks in range(qs - 768, qs - 127, 128):
        tiles.append((ks, None))
    tiles.append((qs, "M4"))
    tiles.append((qs + 128, "M3"))
    return tiles, True


def _build():
    import concourse.bacc as bacc
    import concourse.mybir as mybir
    from concourse.tile import TileContext

    dt = mybir.dt
    F32, BF, F32R = dt.float32, dt.bfloat16, dt.float32r
    AF = mybir.ActivationFunctionType

    nc = bacc.Bacc("TRN2", target_bir_lowering=False, debug=False, num_devices=8)

    xt = nc.dram_tensor("xt", [D, T], BF, kind="ExternalInput")
    wqd = nc.dram_tensor("wqd", [128, 32 * 1024], BF, kind="ExternalInput")
    wkd = nc.dram_tensor("wkd", [128, 32 * 256], BF, kind="ExternalInput")
    wvd = nc.dram_tensor("wvd", [128, 32 * 256], BF, kind="ExternalInput")
    wod = nc.dram_tensor("wod", [128, 8 * 4096], BF, kind="ExternalInput")
    cosd = nc.dram_tensor("cosd", [128, T], BF, kind="ExternalInput")
    sind = nc.dram_tensor("sind", [128, T], BF, kind="ExternalInput")
    maskd = nc.dram_tensor("maskd", [128, 5 * TC], BF, kind="ExternalInput")
    y = nc.dram_tensor("y", [T, D], F32, kind="ExternalOutput")

    xt_v = xt.rearrange("(c p) t -> p c t", p=128)   # [128, 32, 2048]

    sched = [_ktile_schedule(m) for m in range(NTC)]

    with TileContext(nc) as tc:
        with tc.tile_pool(name="persist", bufs=1) as pp:
            KT = pp.tile([128, 2 * T], BF, tag="KT")       # [d, kv*2048+t]
            VS = pp.tile([128, 16 * TC], BF, tag="VS")     # [t%128, tt*256+d']
            QT = pp.tile([128, 8 * T], BF, tag="QT")       # [d, h*2048+t]
            OT = pp.tile([128, 8 * T], BF, tag="OT")       # [d', h*2048+t]
            cos_sb = pp.tile([128, T], BF, tag="cos")
            sin_sb = pp.tile([128, T], BF, tag="sin")
            mask_sb = pp.tile([128, 5 * TC], BF, tag="mask")
            ones_col = pp.tile([128, 1], BF, tag="onec")
            ones_row_f = pp.tile([1, 128], F32, tag="onerf")
            ones_row = pp.tile([1, 128], F32R, tag="oner")
            nc.sync.dma_start(cos_sb[:], cosd[:])
            nc.sync.dma_start(sin_sb[:], sind[:])
            nc.sync.dma_start(mask_sb[:], maskd[:])
            nc.any.memset(ones_col[:], 1.0)
            nc.any.memset(ones_row_f[:], 1.0)
            nc.vector.tensor_copy(ones_row[:], ones_row_f[:])

            def rope_store(dst, ps, tpos, rp):
                """dst = ps*cos + rot64(ps)*sinS for a [128, TC] psum tile."""
                qc = rp.tile([128, TC], BF, tag="ropeqc")
                nc.vector.tensor_mul(qc[:], ps[:], cos_sb[:, tpos:tpos + TC])
                tmp = rp.tile([128, TC], BF, tag="ropetmp")
                nc.vector.tensor_copy(tmp[0:64, :], ps[64:128, :])
                nc.vector.tensor_copy(tmp[64:128, :], ps[0:64, :])
                nc.vector.tensor_mul(tmp[:], tmp[:], sin_sb[:, tpos:tpos + TC])
                nc.vector.tensor_add(dst, qc[:], tmp[:])

            # ---------------- Phase 1: K/V projections ----------------
            with tc.tile_pool(name="wkv", bufs=1) as wkvp, \
                 tc.tile_pool(name="xt1", bufs=2) as xtp1, \
                 tc.tile_pool(name="rope1", bufs=3) as rp1, \
                 tc.tile_pool(name="kvps", bufs=3, space="PSUM") as kvps, \
                 tc.tile_pool(name="vps", bufs=3, space="PSUM") as vps:
                wk_sb = wkvp.tile([128, 32 * 256], BF, tag="wk")
                wv_sb = wkvp.tile([128, 32 * 256], BF, tag="wv")
                nc.sync.dma_start(wk_sb[:], wkd[:])
                nc.sync.dma_start(wv_sb[:], wvd[:])
                for tci in range(NTC):
                    xb = xtp1.tile([128, 32 * TC], BF, tag="xb")
                    nc.sync.dma_start(
                        xb[:].rearrange("p (c t) -> p c t", c=32),
                        xt_v[:, :, tci * TC:(tci + 1) * TC])
                    for kvh in range(2):
                        kp = kvps.tile([128, TC], F32, tag="kp")
                        for Dc in range(32):
                            nc.tensor.matmul(
                                kp[:],
                                wk_sb[:, Dc * 256 + kvh * 128: Dc * 256 + (kvh + 1) * 128],
                                xb[:, Dc * TC:(Dc + 1) * TC],
                                start=(Dc == 0), stop=(Dc == 31))
                        rope_store(KT[:, kvh * T + tci * TC: kvh * T + (tci + 1) * TC],
                                   kp, tci * TC, rp1)
                    for vt in range(2):
                        vp = vps.tile([128, TC], F32, tag="vp")
                        for Dc in range(32):
                            nc.tensor.matmul(
                                vp[:],
                                xb[:, Dc * TC + vt * 128: Dc * TC + vt * 128 + 128],
                                wv_sb[:, Dc * 256:(Dc + 1) * 256],
                                start=(Dc == 0), stop=(Dc == 31))
                        nc.any.tensor_copy(
                            VS[:, (tci * 2 + vt) * TC:(tci * 2 + vt + 1) * TC], vp[:])

            # ---------------- Phase 2: Q projections ----------------
            with tc.tile_pool(name="wqp", bufs=1) as wqp, \
                 tc.tile_pool(name="xt2", bufs=2) as xtp2, \
                 tc.tile_pool(name="rope2", bufs=3) as rp2, \
                 tc.tile_pool(name="qps", bufs=4, space="PSUM") as qps:
                wq_sb = wqp.tile([128, 32 * 1024], BF, tag="wq")
                nc.sync.dma_start(wq_sb[:], wqd[:])
                for tci in range(NTC):
                    xb = xtp2.tile([128, 32 * TC], BF, tag="xb2")
                    nc.sync.dma_start(
                        xb[:].rearrange("p (c t) -> p c t", c=32),
                        xt_v[:, :, tci * TC:(tci + 1) * TC])
                    for h in range(8):
                        qp = qps.tile([128, TC], F32, tag="qp")
                        for Dc in range(32):
                            nc.tensor.matmul(
                                qp[:],
                                wq_sb[:, Dc * 1024 + h * 128: Dc * 1024 + (h + 1) * 128],
                                xb[:, Dc * TC:(Dc + 1) * TC],
                                start=(Dc == 0), stop=(Dc == 31))
                        rope_store(QT[:, h * T + tci * TC: h * T + (tci + 1) * TC],
                                   qp, tci * TC, rp2)

            # ---------------- Phase 3: attention (+ prefetch Wo) ----------------
            with tc.tile_pool(name="wop", bufs=1) as wop:
                wo_sb = wop.tile([128, 8 * 4096], BF, tag="wo")
                nc.sync.dma_start(wo_sb[:], wod[:])
                with tc.tile_pool(name="ep", bufs=6) as ep, \
                     tc.tile_pool(name="np_", bufs=4) as npool, \
                     tc.tile_pool(name="sps", bufs=3, space="PSUM") as sps, \
                     tc.tile_pool(name="ops", bufs=2, space="PSUM") as ops, \
                     tc.tile_pool(name="rps", bufs=2, space="PSUM") as rps, \
                     tc.tile_pool(name="rbps", bufs=1, space="PSUM") as rbps:
                    for qh in range(8):
                        kv = qh // 4
                        for m in range(NTC):
                            tiles, has_g = sched[m]
                            n_acc = len(tiles) + (1 if has_g else 0)
                            qcol = qh * T + m * TC
                            o_ps = ops.tile([128, TC], F32, tag="o")
                            r_ps = rps.tile([1, TC], F32, tag="r")
                            acc = 0
                            for pi in range(0, len(tiles), 2):
                                (ks0, mk0), (ks1, mk1) = tiles[pi], tiles[pi + 1]
                                s_ps = sps.tile([128, 2 * TC], F32, tag="s")
                                for half, ks in ((0, ks0), (1, ks1)):
                                    nc.tensor.matmul(
                                        s_ps[:, half * TC:(half + 1) * TC],
                                        KT[:, kv * T + ks: kv * T + ks + 128],
                                        QT[:, qcol: qcol + TC],
                                        start=True, stop=True)
                                e = ep.tile([128, 2 * TC], BF, tag="e")
                                nc.scalar.activation(e[:], s_ps[:], AF.Exp, scale=SCALE)
                                for half, mk in ((0, mk0), (1, mk1)):
                                    if mk is not None:
                                        mi = MASK_IDX[mk] * TC
                                        nc.vector.tensor_mul(
                                            e[:, half * TC:(half + 1) * TC],
                                            e[:, half * TC:(half + 1) * TC],
                                            mask_sb[:, mi: mi + TC])
                                for half, ks in ((0, ks0), (1, ks1)):
                                    esl = e[:, half * TC:(half + 1) * TC]
                                    st, sp = (acc == 0), (acc == n_acc - 1)
                                    tt = ks // 128
                                    nc.tensor.matmul(
                                        r_ps[:], ones_col[:], esl,
                                        start=st, stop=sp, skip_group_check=True)
                                    nc.tensor.matmul(
                                        o_ps[:],
                                        VS[:, tt * TC + kv * 128: tt * TC + kv * 128 + 128],
                                        esl,
                                        start=st, stop=sp, skip_group_check=True)
                                    acc += 1
                            if has_g:
                                sg = sps.tile([32, TC], F32, tag="s")
                                nc.tensor.matmul(sg[:], KT[:, kv * T: kv * T + 32],
                                                 QT[:, qcol: qcol + TC],
                                                 start=True, stop=True)
                                eg = ep.tile([32, TC], BF, tag="e")
                                nc.scalar.activation(eg[:], sg[:], AF.Exp, scale=SCALE)
                                st, sp = False, True
                                nc.tensor.matmul(r_ps[:], ones_col[0:32, :], eg[:],
                                                 start=st, stop=sp, skip_group_check=True)
                                nc.tensor.matmul(o_ps[:], VS[0:32, kv * 128: kv * 128 + 128],
                                                 eg[:],
                                                 start=st, stop=sp, skip_group_check=True)
                                acc += 1
                            rec_f = npool.tile([1, TC], F32, tag="recf")
                            nc.vector.reciprocal(rec_f[:], r_ps[:])
                            rec = npool.tile([1, TC], F32R, tag="rec")
                            nc.vector.tensor_copy(rec[:], rec_f[:])
                            rb_ps = rbps.tile([128, TC], F32, tag="rb")
                            nc.tensor.matmul(rb_ps[:], ones_row[:], rec[:],
                                             start=True, stop=True)
                            rb_sb = npool.tile([128, TC], F32, tag="rbsb")
                            nc.any.tensor_copy(rb_sb[:], rb_ps[:])
                            nc.vector.tensor_mul(OT[:, qcol: qcol + TC], o_ps[:], rb_sb[:])

                # ---------------- Phase 4: Wo ----------------
                with tc.tile_pool(name="yp", bufs=4) as yp, \
                     tc.tile_pool(name="wps", bufs=4, space="PSUM") as wps:
                    for tt in range(16):
                        for oc in range(8):
                            wpt = wps.tile([128, 512], F32, tag="w")
                            for hc in range(8):
                                nc.tensor.matmul(
                                    wpt[:],
                                    OT[:, hc * T + tt * 128: hc * T + tt * 128 + 128],
                                    wo_sb[:, hc * 4096 + oc * 512: hc * 4096 + (oc + 1) * 512],
                                    start=(hc == 0), stop=(hc == 7))
                            ysb = yp.tile([128, 512], F32, tag="y")
                            nc.any.tensor_copy(ysb[:], wpt[:])
                            nc.sync.dma_start(
                                y[tt * 128:(tt + 1) * 128, oc * 512:(oc + 1) * 512],
                                ysb[:])

    nc.compile()
    return nc


def _host_prep(x, Wq, Wk, Wv, Wo, cos, sin):
    import ml_dtypes
    bf16 = ml_dtypes.bfloat16

    x = np.asarray(x, dtype=np.float32)
    Wq = np.asarray(Wq, dtype=np.float32)
    Wk = np.asarray(Wk, dtype=np.float32)
    Wv = np.asarray(Wv, dtype=np.float32)
    Wo = np.asarray(Wo, dtype=np.float32)
    cos = np.asarray(cos, dtype=np.float32)
    sin = np.asarray(sin, dtype=np.float32)

    xts = [np.ascontiguousarray(x[b].T).astype(bf16) for b in range(B)]

    cosT = np.ascontiguousarray(cos[0, 0].T).astype(bf16)       # [128, T]
    sinS = sin[0, 0].T.copy()
    sinS[:64] = -sinS[:64]
    sinST = np.ascontiguousarray(sinS).astype(bf16)

    i_rel = np.arange(TC)[None, :]
    j_rel = np.arange(128)[:, None]
    M1 = (j_rel > i_rel)
    M1g = M1 | (j_rel < NGLOBAL)
    M2 = (j_rel > i_rel - 128)
    M3 = (i_rel >= j_rel + 128)
    M4 = (i_rel >= j_rel)
    maskpack = np.ascontiguousarray(
        np.concatenate([M1, M1g, M2, M3, M4], axis=1)).astype(bf16)

    def arr_w(w, width):   # [4096, width] -> [128, 32*width]
        return np.ascontiguousarray(
            w.reshape(32, 128, width).transpose(1, 0, 2).reshape(128, 32 * width)
        ).astype(bf16)

    per_g = []
    for g in range(4):
        wq = arr_w(Wq[:, g * 1024:(g + 1) * 1024], 1024)
        wk = arr_w(Wk[:, g * 256:(g + 1) * 256], 256)
        wv = arr_w(Wv[:, g * 256:(g + 1) * 256], 256)
        wo = np.ascontiguousarray(
            Wo[g * 1024:(g + 1) * 1024, :].reshape(8, 128, 4096)
            .transpose(1, 0, 2).reshape(128, 8 * 4096)).astype(bf16)
        per_g.append((wq, wk, wv, wo))

    in_maps = []
    for c in range(8):
        b, g = divmod(c, 4)
        wq, wk, wv, wo = per_g[g]
        in_maps.append({
            "xt": xts[b], "wqd": wq, "wkd": wk, "wvd": wv, "wod": wo,
            "cosd": cosT, "sind": sinST, "maskd": maskpack,
        })
    return in_maps


def kernel(x, Wq, Wk, Wv, Wo, cos, sin, mask):
    global LAST_RESULT
    from concourse.bass_utils import run_bass_kernel_spmd

    if "nc" not in _CACHE:
        _CACHE["nc"] = _build()
    nc = _CACHE["nc"]

    in_maps = _host_prep(x, Wq, Wk, Wv, Wo, cos, sin)
    res = run_bass_kernel_spmd(nc, in_maps, list(range(8)), **RUN_KWARGS)
    LAST_RESULT = res

    y = np.zeros((B, T, D), dtype=np.float32)
    for c in range(8):
        b, _ = divmod(c, 4)
        y[b] += res.results[c]["y"].reshape(T, D)
    return y
